# revision 13
# baseline (speedup 1.0000x reference)
"""Trainium2 Bass kernel for nn_Actor (diagonal complex LRU, last-step output).

Math: the reference scans x_t = lam*x_{t-1} + (gamma*B) u_t over L=2048 steps
and keeps y[:, -1, :].  The last state collapses to
    x_L[n] = sum_k lam[n]^k * (Bhat @ u_{L-1-k})[n]
Because |lam| <= 0.99 by construction (LRU stable init), the sum truncates:
modes are sorted by |lam| (host-side permutation of the diagonal; the output
is permutation invariant); the top 128 modes get 2 time-tiles of 128 steps
(K=256; float64-validated truncation error 3.0e-3 vs the 2e-2 gate) and the
bottom 128 modes 1 tile.  Per-core device work:
    v[n, b, h] = sum_k W[k, n] * u[b, L-1-k, k]     (TensorE, PSUM accum;
                 tile0 bf16, tile1 plain fp8)
    q pairs    = B-products of v_re / v_nim         (VectorE bf16, 2-slot TTs)
    ypsum[o,b,hc] = C-projection of q, h-chunked    (TensorE, contracts n;
                 accumulates 32-wide h-chunks so the final reduce is small)
    y[o, b]    = sum_hc ypsum + D u_last            (VectorE reduce + add)
Sign trick: W_im is shipped NEGATED, and P1 carries [-bim, bre, bim] per
half, which makes all four complex-product terms take POSITIVE +Cre/+Cim
projections -- no negated C copies shipped and no on-device negation.
All DMAs ride the sync HWDGE ring in priority order (u+W first); no SWDGE
(gpsimd) DMAs -- their descriptor rings degrade SDMA engines 14/15 and
create a multi-microsecond completion tail.  Dummy warm-up matmuls (gated
by a vector memset, the earliest-ready engine) keep the PE HAM warm while
the DMAs land.

Sharding: data-parallel over batch (64 -> 8 per core) on 8 NeuronCores,
no collectives; host concatenates per-core outputs.
"""

import os
import sys

# reset cores at runtime init: recovers wedged exec units
# (NRT_EXEC_UNIT_UNRECOVERABLE) if a prior run left the device bad;
# harmless no-op when the runtime is already initialized elsewhere
os.environ.setdefault("NEURON_RT_RESET_CORES", "1")

sys.path.insert(0, "/opt/trn_rl_repo")

import numpy as np
from ml_dtypes import bfloat16 as np_bf16
from ml_dtypes import float8_e4m3 as np_f8

import concourse.bass as bass
import concourse.tile as tile
from concourse import bacc, mybir
from concourse.bass_utils import run_bass_kernel_spmd

B, L, H, O, N = 64, 2048, 128, 128, 256
NCORES = 8
BS = B // NCORES  # 8 batches per core
KT0 = 2  # time tiles for the top-half modes (K=256 steps)
F32 = mybir.dt.float32
BF16 = mybir.dt.bfloat16
F8 = mybir.dt.float8e4
N_WARM = 30  # PE ramp-up dummies while DMAs land (small: free dim 128)
HC = 32  # h-chunk width for the C-projection (shrinks the final reduce)


def build():
    nc = bacc.Bacc("TRN2", target_bir_lowering=False, debug=False)

    # uwa: W0 re (both halves) + u tile0 -- the critical first DMA; the
    # re-component mains (including the whole h1-epilogue feeder chain)
    # depend only on this.  uwb: W0 negated-im (both halves).
    uwa_d = nc.dram_tensor("uwa", [128, 10 * 128], BF16, kind="ExternalInput")
    uwb_d = nc.dram_tensor("uwb", [128, 2 * 128], BF16, kind="ExternalInput")
    # b1: [w1_re | w1_nim | u tile1 (8)] fp8
    b1_d = nc.dram_tensor("b1", [128, 10 * 128], F8, kind="ExternalInput")
    # P1: per half: [-bim, bre, bim]  (re-TT reads slots 0-1, im-TT slots 1-2)
    p1_d = nc.dram_tensor("P1", [128, 6 * 128], BF16, kind="ExternalInput")
    # P2: cret_h0, cret_h1, cimt_h0, cimt_h1, dT, ulT  (all POSITIVE signs)
    p2_d = nc.dram_tensor("P2", [128, 5 * 128 + BS], BF16, kind="ExternalInput")
    out_d = nc.dram_tensor("out", [O, BS], F32, kind="ExternalOutput")

    mult = mybir.AluOpType.mult
    add = mybir.AluOpType.add

    with tile.TileContext(nc) as tc:
        with (
            tc.tile_pool(name="cp", bufs=1) as cp,
            tc.tile_pool(name="psum", bufs=1, space=bass.MemorySpace.PSUM) as pp,
        ):
            # warm tile memset first: it unblocks the PE ramp-up dummies;
            # vector is the earliest-ready engine after the runtime preamble
            warm = cp.tile([128, 128], BF16, tag="warm")
            nc.vector.memset(warm[:], 0.125)

            # ---- DMAs: one HWDGE ring (sync), FIFO priority order --------
            uwa = cp.tile([128, 10, 128], BF16, tag="uwa")
            nc.sync.dma_start(uwa[:], uwa_d.reshape([128, 10, 128])[:])
            uwb = cp.tile([128, 2, 128], BF16, tag="uwb")
            nc.sync.dma_start(uwb[:], uwb_d.reshape([128, 2, 128])[:])
            p1 = cp.tile([128, 6, 128], BF16, tag="p1")
            nc.sync.dma_start(p1[:], p1_d.reshape([128, 6, 128])[:])
            b1 = cp.tile([128, 10, 128], F8, tag="b1")
            nc.sync.dma_start(b1[:], b1_d.reshape([128, 10, 128])[:])
            p2 = cp.tile([128, 5 * 128 + BS], BF16, tag="p2")
            nc.sync.dma_start(p2[:], p2_d[:])

            def w0(c, hf):  # c: 0=re 1=neg-im
                return uwa[:, hf, :] if c == 0 else uwb[:, hf, :]

            def u0(bh):
                return uwa[:, 2 + bh * 4 : 2 + (bh + 1) * 4, :]

            def w1(c):
                return b1[:, c, :]

            def u1(bh):
                return b1[:, 2 + bh * 4 : 2 + (bh + 1) * 4, :]

            cret = [p2[:, hf * 128 : (hf + 1) * 128] for hf in range(2)]
            cimt = [p2[:, (2 + hf) * 128 : (3 + hf) * 128] for hf in range(2)]
            dT = p2[:, 4 * 128 : 5 * 128]
            ulT = p2[:, 5 * 128 : 5 * 128 + BS]

            # ---- PSUM: v accumulators (c: 0=re, 1=w=-im) ------------------
            pv = {
                (c, hf): pp.tile(
                    [128, BS, H], F32, tag=f"pv{c}{hf}", name=f"pv{c}{hf}"
                )
                for c in range(2)
                for hf in range(2)
            }

            # ---- PE warm-up: keep the clock ramped while DMAs land --------
            for _ in range(N_WARM):
                nc.tensor.matmul(
                    pv[(0, 0)][:, 0:1, :], warm[:], warm[:], start=True, stop=True
                )

            def bsl(ap, bh):
                return ap[:, bh * 4 : (bh + 1) * 4, :]

            # ---- main contraction ----------------------------------------
            # order = DMA-landing order: all re tile0 (h0 then h1 -- only
            # uwa needed), then w tile0 (uwb), then the fp8 tile1 (b1);
            # pv_re_1 completes first and feeds the scalar-copy chain
            for hf in range(2):
                for bh in range(2):
                    nc.tensor.matmul(
                        bsl(pv[(0, hf)], bh), w0(0, hf), u0(bh),
                        start=True, stop=hf == 1,
                    )
            for hf in range(2):
                for bh in range(2):
                    nc.tensor.matmul(
                        bsl(pv[(1, hf)], bh), w0(1, hf), u0(bh),
                        start=True, stop=hf == 1,
                    )
            for c in range(2):
                for bh in range(2):
                    nc.tensor.matmul(
                        bsl(pv[(c, 0)], bh), w1(c), u1(bh),
                        start=False, stop=True,
                    )

            # ---- epilogue -------------------------------------------------
            # PSUM reuse: ypsum takes pv_re_1's banks (copied-out first)
            ypsum = pp.tile([128, BS, HC], F32, tag="pv01", name="ypsum")

            sv = {}

            def sv_copy(c, hf):
                # all copies on scalar: a serial 1.15us/copy feeder that runs
                # in lockstep with the vector TT chain (1.2us/TT-pair)
                t = cp.tile([128, BS, H], BF16, tag=f"sv{c}{hf}", name=f"sv{c}{hf}")
                nc.scalar.copy(t[:], pv[(c, hf)][:])
                sv[(c, hf)] = t

            ystate = [None]  # ypsum sits in one bank -> one accumulation group

            def qproj(q_ap, lhs, bh, last=False):
                # q_ap: [128, 4, 128] rhs source for this bh; h-chunked accum
                for ch in range(H // HC):
                    nc.tensor.matmul(
                        ypsum[:, bh * 4 : (bh + 1) * 4, :],
                        lhs,
                        q_ap[:, :, ch * HC : (ch + 1) * HC],
                        start=(ystate[0] is None),
                        stop=last and ch == H // HC - 1,
                    )
                    ystate[0] = True

            def tt_pair(c, hf, bh_split=None, last=False):
                # one 2-slot TT: slot0 -> cimt[hf], slot1 -> cret[hf]
                # re-TT (c=0): in1 slots (3hf+0, 3hf+1) = (-bim, bre)
                # im-TT (c=1): in1 slots (3hf+1, 3hf+2) = (bre, bim)
                bhs = (0, 1) if bh_split is None else (bh_split,)
                bw = 4 * len(bhs)
                b0 = bhs[0] * 4
                q = cp.tile(
                    [128, 2, BS, H], BF16, tag=f"q{c}{hf}",
                    name=f"q{c}{hf}b{bh_split}",
                )
                s0 = 3 * hf + c
                in0 = sv[(c, hf)][:, None, b0 : b0 + bw, :].broadcast_to(
                    [128, 2, bw, H]
                )
                in1 = p1[:, s0 : s0 + 2, None, :].broadcast_to([128, 2, bw, H])
                nc.vector.tensor_tensor(q[:, :, b0 : b0 + bw, :], in0, in1, mult)
                for slot, lhs in ((0, cimt[hf]), (1, cret[hf])):
                    for bh in bhs:
                        qproj(
                            q[:, slot, bh * 4 : (bh + 1) * 4, :], lhs, bh,
                            last=last and slot == 1 and bh == bhs[-1],
                        )

            # copies in psum-readiness order; TTs trail one copy behind
            sv_copy(0, 1)
            sv_copy(1, 1)
            tt_pair(0, 1)
            sv_copy(0, 0)
            # D u_last folded into ypsum's h-chunk accumulation group: its
            # [O, BS] product lands in chunk column 0 and the final reduce
            # absorbs it -- no separate psum tile, no final vector add
            nc.tensor.matmul(
                ypsum[:, :, 0:1], dT, ulT, start=False, stop=False,
                skip_group_check=True,
            )
            tt_pair(1, 1)
            sv_copy(1, 0)
            tt_pair(0, 0)
            # last group split by batch-half so bh0's projection matmuls
            # overlap bh1's TT instead of trailing the whole pair
            tt_pair(1, 0, bh_split=0)
            tt_pair(1, 0, bh_split=1, last=True)

            # ---- final reduce over h-chunks (D already folded) + store ----
            out_sb = cp.tile([O, BS], F32, tag="out_sb")
            nc.vector.tensor_reduce(out_sb[:], ypsum[:], mybir.AxisListType.X, add)
            nc.sync.dma_start(out_d[:, :], out_sb[:])

    nc.compile()
    return nc


_NC_CACHE = {}


def _get_nc():
    if "nc" not in _NC_CACHE:
        _NC_CACHE["nc"] = build()
    return _NC_CACHE["nc"]


def _plan(inputs):
    """Host-side: mode sort, lam-power tables, param packing (float64)."""
    nu = np.asarray(inputs["nu_log"], np.float64)
    th = np.asarray(inputs["theta_log"], np.float64)
    gm = np.asarray(inputs["gamma_log"], np.float64)
    lam_abs = np.exp(-np.exp(nu))
    order = np.argsort(-lam_abs)  # descending |lam|
    sl = lam_abs[order]
    # fail loudly if the input distribution ever changes enough that the
    # hardcoded K=256/128 truncation would break the 2e-2 gate
    assert sl[0] ** (128 * KT0) < 0.09, "top-half modes decay too slowly"
    assert sl[128] ** 128 < 0.09, "bottom-half modes decay too slowly"

    lam = np.exp(-np.exp(nu[order]) + 1j * np.exp(th[order]))
    ks = np.arange(128, dtype=np.float64)[:, None]

    def wslot(j, hf):  # [128k, 128n] complex
        base = lam[hf * 128 : (hf + 1) * 128]
        return base ** (128.0 * j + ks)

    Bre = np.asarray(inputs["B_re"], np.float64)[order]
    Bim = np.asarray(inputs["B_im"], np.float64)[order]
    g = np.exp(gm[order])[:, None]
    bre, bim = Bre * g, Bim * g
    Cre = np.asarray(inputs["C_re"], np.float64)[:, order]
    Cim = np.asarray(inputs["C_im"], np.float64)[:, order]
    D = np.asarray(inputs["D"], np.float64)

    P1 = np.zeros((128, 6, 128), np.float64)
    P2 = np.zeros((128, 5 * 128 + BS), np.float64)
    for hf in range(2):
        s = hf * 128
        P1[:, 3 * hf + 0] = -bim[s : s + 128]
        P1[:, 3 * hf + 1] = bre[s : s + 128]
        P1[:, 3 * hf + 2] = bim[s : s + 128]
        P2[:, hf * 128 : (hf + 1) * 128] = Cre[:, s : s + 128].T
        P2[:, (2 + hf) * 128 : (3 + hf) * 128] = Cim[:, s : s + 128].T
    P2[:, 4 * 128 : 5 * 128] = D.T
    return {"wslot": wslot, "P1": P1, "P2": P2}


def _make_in_maps(inputs, plan=None):
    if plan is None:
        plan = _plan(inputs)
    wslot = plan["wslot"]
    u = np.asarray(inputs["dynamics_disturbance_time_window"], np.float32)
    urev = np.ascontiguousarray(u[:, ::-1, :][:, : KT0 * 128, :]).reshape(
        B, KT0, 128, H
    )

    # W slot0 (both halves) and slot1 (top half), im NEGATED
    w0re = np.zeros((128, 2, 128), np.float64)
    w0nim = np.zeros((128, 2, 128), np.float64)
    for hf in range(2):
        wj = wslot(0, hf)
        w0re[:, hf] = wj.real
        w0nim[:, hf] = -wj.imag
    w1 = wslot(1, 0)

    in_maps = []
    for c in range(NCORES):
        ub = urev[c * BS : (c + 1) * BS]  # [BS, KT0, 128, H]
        uwa = np.zeros((128, 10, 128), np.float32)
        uwa[:, 0:2] = w0re
        uwa[:, 2:10] = ub[:, 0].transpose(1, 0, 2)
        b1 = np.zeros((128, 10, 128), np.float32)
        b1[:, 0] = w1.real
        b1[:, 1] = -w1.imag
        b1[:, 2:10] = ub[:, 1].transpose(1, 0, 2)
        P2 = plan["P2"].copy()
        P2[:, 5 * 128 : 5 * 128 + BS] = (
            u[c * BS : (c + 1) * BS, L - 1, :].astype(np.float64).T
        )
        in_maps.append(
            {
                "uwa": uwa.astype(np_bf16).reshape(128, 10 * 128),
                "uwb": w0nim.astype(np_bf16).reshape(128, 2 * 128),
                "b1": b1.astype(np_f8).reshape(128, 10 * 128),
                "P1": plan["P1"].astype(np_bf16).reshape(128, 6 * 128),
                "P2": P2.astype(np_bf16),
            }
        )
    return in_maps


def _ensure_profile_hook():
    """The agent image's antenv lacks axon_hooks; shim it and register the
    ctypes NTFF hook so run_bass_kernel_spmd(trace=True) can profile."""
    import types

    if "antenv.axon_hooks" in sys.modules:
        return
    mod = types.ModuleType("antenv.axon_hooks")
    mod._hook = None
    mod.set_axon_ntff_profile_hook = lambda h: setattr(mod, "_hook", h)
    mod.get_axon_ntff_profile_hook = lambda: mod._hook
    sys.modules["antenv.axon_hooks"] = mod
    try:
        from trn_agent_boot.trn_boot import _ntff_profile_via_ctypes

        mod._hook = _ntff_profile_via_ctypes("/opt/axon/libaxon_pjrt.so")
    except Exception as e:
        print(f"profile hook setup failed: {e}", file=sys.stderr)


def run(inputs, trace=False, tmpdir=None):
    if trace:
        _ensure_profile_hook()
    plan = _plan(inputs)
    nc = _get_nc()
    in_maps = _make_in_maps(inputs, plan)
    res = run_bass_kernel_spmd(
        nc, in_maps, list(range(NCORES)), trace=trace, tmpdir=tmpdir
    )
    out = np.concatenate(
        [np.asarray(res.results[i]["out"]).T for i in range(NCORES)], axis=0
    )
    return out.astype(np.float32), res


def kernel(**inputs):
    out, _ = run(inputs, trace=False)
    return out
